# revision 1
# baseline (speedup 1.0000x reference)
"""MoE (top-2, 8 experts, SwiGLU + shared expert) on 8 TRN2 NeuronCores.

Expert-parallel, bf16 compute.  Host computes the (tiny) router, gathers
each expert's tokens into a padded [C, DIM] block (pre-scaled by router
score), appends the core's 1/8 shared-expert token shard, and ships core e
ONE feature-major activation tensor xT [DIM, C+S] (bf16) plus its expert
weights (w1/w3 column-interleaved w13 [DIM, 2*HIDDEN], w2 [HIDDEN, DIM])
and the replicated shared-expert weights.  The core runs two dense SwiGLU
MLPs feature-major and writes ONE packed output yT [DIM, C+S] (bf16);
host scatter-adds the routed columns into the shared-expert output.

Device schedule (raw Bass, manual semaphores — at most one inline sync
wait / one then_inc per instruction):
  - weights stream in PASS-granular blocks: one DMA moves [1024, 256]
    (all 8 k-tiles of a 256-wide column block) into one SBUF slot laid
    out [128, 8*256], so the whole program is ~25 input DMAs (the SP
    DGE issue pipeline, 565ns/DMA, stays far off the critical path).
  - phase order: routed-A, shared-A, routed-B, shared-B.  A-phases
    produce g = silu(h1)*h3 (bf16) per hidden tile; B-phases contract
    g with w2 and stream m-tiles of yT out.
  - PSUM double-banking: A/B routed passes use 4-bank sets (2 m-tiles x
    2 token chunks) alternating {0-3}/{4-7}; shared passes use 2-bank
    sets rotating through all 8.

Engine roles:
  sync  (SP) : x + weight streaming DMAs (FIFO, 4-slot ring)
  tensor(PE) : all matmuls (bf16, 1 row/cycle)
  scalar(ACT): silu eviction PSUM->SBUF; output DMAs
  vector(DVE): g = silu(h1)*h3 multiply; PSUM->SBUF output copies
"""

from contextlib import ExitStack

import numpy as np

import concourse.bass as bass
import concourse.mybir as mybir

DIM = 1024
HIDDEN = 1024
NUM_EXPERTS = 8
TOP_K = 2
N_CORES = 8
P = 128
KT = DIM // P

DT = mybir.dt.bfloat16
W_RING = 8    # weight-slot ring depth (deep prefetch smooths chip HBM load;
              # 16 measured equivalent — contention is sustained-BW-limited)
S_RING = 4    # silu scratch ring
O_RING = 4    # output tile ring
NSEM_W = 16   # weight-DMA completion sem ring (must be >= W_RING)
NSEM_OD = 4   # output-DMA completion sem ring
ACT_FUNC = mybir.ActivationFunctionType.Silu


def _chunks(total, maxc=512):
    """Split `total` into nearly-equal chunks of at most `maxc` columns."""
    if total <= maxc:
        return [(0, total)]
    n = (total + maxc - 1) // maxc
    h = ((total + n - 1) // n + 15) // 16 * 16
    out, off = [], 0
    while off + h < total:
        out.append((off, h))
        off += h
    out.append((off, total - off))
    return out


class Plan:
    """Per-engine instruction streams with planned semaphore counters."""

    ENGINES = ("sync", "tensor", "scalar", "vector")

    def __init__(self):
        self.streams = {e: [] for e in self.ENGINES}
        self.cnt = {}
        self._waited = {}

    def wait(self, eng, sem, val):
        val = int(val)
        if val <= 0 or self._waited.get((eng, sem), 0) >= val:
            return
        self._waited[(eng, sem)] = val
        self.streams[eng].append(("wait", sem, val))

    def op(self, eng, fn, incs=()):
        self.streams[eng].append(("op", fn, tuple(incs)))
        for s, v in incs:
            self.cnt[s] = self.cnt.get(s, 0) + v


def build_program(C, S, act_func=ACT_FUNC, repeat=1):
    XW = C + S
    rch = _chunks(C)          # routed token chunks (local cols)
    sch = _chunks(S)          # shared token chunks
    assert len(rch) <= 2 and len(sch) <= 2

    nc = bass.Bass()
    tens = {}
    tens["xT"] = nc.declare_dram_parameter("xT", [DIM, XW], DT, isOutput=False)
    tens["w13"] = nc.declare_dram_parameter("w13", [DIM, 2 * HIDDEN], DT,
                                            isOutput=False)
    tens["w2"] = nc.declare_dram_parameter("w2", [HIDDEN, DIM], DT,
                                           isOutput=False)
    tens["w13s"] = nc.declare_dram_parameter("w13s", [DIM, 2 * HIDDEN], DT,
                                             isOutput=False)
    tens["w2s"] = nc.declare_dram_parameter("w2s", [HIDDEN, DIM], DT,
                                            isOutput=False)
    tens["yT"] = nc.declare_dram_parameter("yT", [DIM, XW], DT, isOutput=True)

    cwmax = max(cw for _, cw in rch + sch)

    plan = Plan()
    st = {"pass_idx": 0, "dma_idx": 0, "od_idx": 0, "s_idx": 0, "o_idx": 0,
          "bank_rel": [None] * 8, "s_rel": [None] * S_RING,
          "o_rel": [None] * O_RING, "slot_done": [0] * W_RING}

    def weight_dma(wname, m0, mcols, n_passes=1):
        """One weight block [1024, mcols] -> slot [128, 8*mcols], feeding the
        next `n_passes` PE passes.  mcols=256 keeps DMA row segments at 512B
        (the DMA engine's efficiency threshold)."""
        d = st["dma_idx"]
        slot = d % W_RING
        if d >= W_RING:
            plan.wait("sync", "mm", st["slot_done"][slot])

        def fn(e, _slot=slot, _nm=wname, _m0=m0, _mc=mcols):
            src = tens[_nm][:, _m0:_m0 + _mc].rearrange(
                "(kk p) c -> p kk c", kk=KT)
            return e.dma_start(out=tens[f"wt{_slot}"][:, :KT * _mc], in_=src)

        wsem = f"w{d % NSEM_W}"
        wval = plan.cnt.get(wsem, 0) + 16
        plan.op("sync", fn, incs=((wsem, 16),))
        st["slot_done"][slot] = st["pass_idx"] + n_passes
        st["dma_idx"] += 1
        return slot, (wsem, wval)

    def pe_pass(slot, wsem_val, rhs_base, chunks, bankset, g_wait=None,
                incremental_x=False, n_ml=2, slot_mcols=None, ml_off=0):
        """The 8-k-tile matmul burst of one pass (n_ml m-tiles x chunks).

        slot_mcols: column-block width per k-tile in the weight slot (the
        DMA'd block may hold more m-tiles than this pass consumes);
        ml_off: first m-sub-tile of the block this pass covers."""
        p = st["pass_idx"]
        if wsem_val is not None:
            wsem, wval = wsem_val
            plan.wait("tensor", wsem, wval)
        if not incremental_x:
            for kk in range(KT):
                plan.wait("tensor", f"x{kk}", 16)
        ncn = len(chunks)
        mcols = slot_mcols if slot_mcols is not None else n_ml * P
        n_mm = KT * n_ml * ncn
        i = 0
        for kk in range(KT):
            if incremental_x:
                plan.wait("tensor", f"x{kk}", 16)
                plan.wait("tensor", f"wk{kk}", 16)
            if g_wait is not None:
                plan.wait("tensor", "g", g_wait(kk))
            for ml in range(n_ml):
                for ci, (c0, cw) in enumerate(chunks):
                    b = bankset[ml * ncn + ci]
                    if kk == 0 and st["bank_rel"][b] is not None:
                        rs, rv = st["bank_rel"][b]
                        plan.wait("tensor", rs, rv)
                    i += 1
                    incs = (("mm", 1),) if i == n_mm else ()

                    def mmop(e, _b=b, _slot=slot, _kk=kk, _ml=ml_off + ml,
                             _c0=c0, _cw=cw, _rb=rhs_base, _mc=mcols):
                        return e.matmul(
                            tens[f"pb{_b}"][:, :_cw],
                            lhsT=tens[f"wt{_slot}"][:, _kk * _mc + _ml * P:
                                                    _kk * _mc + (_ml + 1) * P],
                            rhs=tens["xg"][:, _kk * XW + _rb + _c0:
                                           _kk * XW + _rb + _c0 + _cw],
                            start=(_kk == 0), stop=(_kk == KT - 1),
                            skip_group_check=True)

                    plan.op("tensor", mmop, incs=incs)
        st["pass_idx"] += 1

    def a_pass(wname, j, xbase, chunks, bankset, incremental_x=False):
        """Phase-A pass j: h-tile j of silu(x@w1)*(x@w3) -> g columns."""
        if incremental_x:
            # First pass of the program: x tiles stream from the (idle)
            # ACT/DVE DMA queues while SP issues a k-split weight block,
            # so the PE starts after ~2 tiles instead of after the whole
            # 1.3MB x + 512KB weight transfers.
            slot = st["dma_idx"] % W_RING
            for kk in range(KT):
                def xl(e, _kk=kk):
                    return e.dma_start(
                        out=tens["xg"][:, _kk * XW:(_kk + 1) * XW],
                        in_=tens["xT"][_kk * P:(_kk + 1) * P, :])

                plan.op("scalar", xl, incs=((f"x{kk}", 16),))

                def wl(e, _slot=slot, _nm=wname, _kk=kk, _m0=j * 256):
                    return e.dma_start(
                        out=tens[f"wt{_slot}"][:, _kk * 256:(_kk + 1) * 256],
                        in_=tens[_nm][_kk * P:(_kk + 1) * P, _m0:_m0 + 256])

                plan.op("sync", wl, incs=((f"wk{kk}", 16),))
            st["slot_done"][slot] = st["pass_idx"] + 1
            st["dma_idx"] += 1
            pe_pass(slot, None, xbase, chunks, bankset, incremental_x=True)
        else:
            slot, wv = weight_dma(wname, j * 256, 256)
            pe_pass(slot, wv, xbase, chunks, bankset)
        pdone = st["pass_idx"]  # mm value when this pass completes
        ncn = len(chunks)
        for ci, (c0, cw) in enumerate(chunks):
            st["s_idx"] += 1
            s_slot = st["s_idx"] % S_RING
            plan.wait("scalar", "mm", pdone)
            if st["s_rel"][s_slot] is not None:
                rs, rv = st["s_rel"][s_slot]
                plan.wait("scalar", rs, rv)

            def silu(e, _s=s_slot, _b=bankset[ci], _cw=cw):
                return e.activation(tens[f"s{_s}"][:, :_cw],
                                    tens[f"pb{_b}"][:, :_cw], act_func)

            plan.op("scalar", silu, incs=(("s", 1),))
            st["bank_rel"][bankset[ci]] = ("s", plan.cnt["s"])
            s_need = plan.cnt["s"]
            plan.wait("vector", "mm", pdone)
            plan.wait("vector", "s", s_need)
            b3 = bankset[ncn + ci]

            def mul(e, _j=j, _s=s_slot, _b=b3, _xb=xbase, _c0=c0, _cw=cw):
                col = _j * XW + GHALF + _xb + _c0
                return e.tensor_mul(tens["xg"][:, col:col + _cw],
                                    tens[f"s{_s}"][:, :_cw],
                                    tens[f"pb{_b}"][:, :_cw])

            plan.op("vector", mul, incs=(("g", 1),))
            st["bank_rel"][b3] = ("g", plan.cnt["g"])
            st["s_rel"][s_slot] = ("g", plan.cnt["g"])

    def b_pass(slot, wv, m, ml_off, gbase, chunks, bankset, g_cnt_base,
               ybase, yw):
        """Phase-B pass: single m-tile m of g @ w2 -> yT[:, ybase:ybase+yw].

        One m-tile per pass (each tile's PSUM->SBUF copy + output DMA
        overlaps the next tile's matmuls; the end-of-program drain is one
        small tile's chain), but weight blocks are DMA'd 256 cols at a
        time (512B descriptors) and shared by the pass pair."""
        ncn = len(chunks)
        pe_pass(slot, wv, GHALF + gbase, chunks, bankset,
                g_wait=lambda kk: g_cnt_base + ncn * (kk + 1), n_ml=1,
                slot_mcols=256, ml_off=ml_off)
        pdone = st["pass_idx"]
        st["o_idx"] += 1
        o_slot = st["o_idx"] % O_RING
        plan.wait("vector", "mm", pdone)
        if st["o_rel"][o_slot] is not None:
            rs, rv = st["o_rel"][o_slot]
            plan.wait("vector", rs, rv)
        for ci, (c0, cw) in enumerate(chunks):
            b = bankset[ci]

            def cp(e, _o=o_slot, _b=b, _c0=c0, _cw=cw):
                return e.tensor_copy(tens[f"ot{_o}"][:, _c0:_c0 + _cw],
                                     tens[f"pb{_b}"][:, :_cw])

            plan.op("vector", cp, incs=(("o", 1),))
            st["bank_rel"][b] = ("o", plan.cnt["o"])
        o_need = plan.cnt["o"]
        plan.wait("scalar", "o", o_need)
        odsem = f"od{st['od_idx'] % NSEM_OD}"
        odval = 16 * (st["od_idx"] // NSEM_OD + 1)
        st["od_idx"] += 1
        st["o_rel"][o_slot] = (odsem, odval)

        def odma(e, _o=o_slot, _m=m, _yb=ybase, _yw=yw):
            return e.dma_start(
                out=tens["yT"][_m * P:(_m + 1) * P, _yb:_yb + _yw],
                in_=tens[f"ot{_o}"][:, :_yw])

        plan.op("scalar", odma, incs=((odsem, 16),))

    with ExitStack() as ctx:
        def sb(name, shape, dt):
            tens[name] = ctx.enter_context(nc.sbuf_tensor(name, shape, dt))

        # xg holds x tiles (first KT*XW cols) and g tiles (next KT*XW)
        GHALF = KT * XW
        sb("xg", [P, 2 * KT * XW], DT)
        for r in range(W_RING):
            sb(f"wt{r}", [P, 2048], DT)
        for r in range(S_RING):
            sb(f"s{r}", [P, cwmax], DT)
        for r in range(O_RING):
            sb(f"ot{r}", [P, max(C, S)], DT)
        for b in range(8):
            tens[f"pb{b}"] = ctx.enter_context(
                nc.psum_tensor(f"pb{b}", [P, 512], mybir.dt.float32))

        # ---- phases (x loads are interleaved into the first A pass) ----
        # (A PE warm-up — discarded matmuls on zeroed scratch during the
        # initial DMA wait to pre-ramp the clock — was tried and measured
        # only 0.09us in-model; dropped as below the noise floor.)
        r4 = [(0, 1, 2, 3), (4, 5, 6, 7)]       # routed 4-bank sets
        s2 = [(0, 1), (2, 3), (4, 5), (6, 7)]   # shared 2-bank sets
        for _rep in range(repeat):
            g0 = plan.cnt.get("g", 0)
            for j in range(KT):          # routed A: 8 passes
                a_pass("w13", j, 0, rch, r4[j % 2],
                       incremental_x=(_rep == 0 and j == 0))
            g_routed_end = plan.cnt.get("g", 0)
            for j in range(KT):          # shared A: 8 passes
                a_pass("w13s", j, C, sch, s2[j % 4])
            for q in range(DIM // 256):  # routed B: 4 blocks, 8 m-passes
                slot, wv = weight_dma("w2", q * 256, 256, n_passes=2)
                for ml in range(2):
                    m = 2 * q + ml
                    b_pass(slot, wv, m, ml, 0, rch, s2[m % 4], g0, 0, C)
            for q in range(DIM // 256):  # shared B: 4 blocks, 8 m-passes
                slot, wv = weight_dma("w2s", q * 256, 256, n_passes=2)
                for ml in range(2):
                    m = 2 * q + ml
                    b_pass(slot, wv, m, ml, C, sch, (m % 8,),
                           g_routed_end, C, S)

        for r in range(NSEM_OD):
            if plan.cnt.get(f"od{r}", 0):
                plan.wait("scalar", f"od{r}", plan.cnt[f"od{r}"])

        # ---- emit ----
        with ExitStack() as sem_ctx:
            sems = {}
            for name in plan.cnt:
                sems[name] = sem_ctx.enter_context(nc.semaphore(f"sem_{name}"))

            with nc.Block() as block:
                def runner(stream):
                    def run(e):
                        for item in stream:
                            if item[0] == "wait":
                                _, s, v = item
                                e.wait_ge(sems[s], v)
                            else:
                                _, fn, incs = item
                                inst = fn(e)
                                rest = list(incs)
                                if rest and inst is not None:
                                    s, v = rest.pop(0)
                                    inst.then_inc(sems[s], v)
                                for s, v in rest:
                                    e.sem_inc(sems[s], v)
                    return run

                block.sync(runner(plan.streams["sync"]))
                block.tensor(runner(plan.streams["tensor"]))
                block.scalar(runner(plan.streams["scalar"]))
                block.vector(runner(plan.streams["vector"]))
    return nc


def _interleave_w13(w1e, w3e, np_dt):
    out = np.empty((DIM, 2 * HIDDEN), dtype=np.float32)
    v = out.reshape(DIM, HIDDEN // P, 2, P)
    v[:, :, 0, :] = w1e.reshape(DIM, HIDDEN // P, P)
    v[:, :, 1, :] = w3e.reshape(DIM, HIDDEN // P, P)
    return out.astype(np_dt)


def route(xt, gate_w):
    logits = (xt @ gate_w.T).astype(np.float32)
    m = logits.max(axis=1, keepdims=True)
    e = np.exp(logits - m)
    scores = (e / e.sum(axis=1, keepdims=True)).astype(np.float32)
    sel = np.argsort(-scores, axis=1, kind="stable")[:, :TOP_K].astype(np.int32)
    top_scores = np.take_along_axis(scores, sel, axis=1)
    sel_flat = sel.reshape(-1)
    order = np.argsort(sel_flat, kind="stable")
    token_idx = (order // TOP_K).astype(np.int64)
    eid = sel_flat[order]
    scores_sorted = top_scores.reshape(-1)[order]
    return token_idx, eid, scores_sorted


def kernel(x, gate_w, w1, w2, w3, w1s, w2s, w3s, _run=None):
    x = np.asarray(x, dtype=np.float32)
    bs, slen, dim = x.shape
    N = bs * slen
    xt = np.ascontiguousarray(x.reshape(N, dim))
    S = N // N_CORES

    token_idx, eid, scores_sorted = route(xt, np.asarray(gate_w, np.float32))

    counts = np.bincount(eid, minlength=NUM_EXPERTS)
    C = int(max(256, ((counts.max() + 7) // 8) * 8))

    np_dt = mybir.dt.np(DT)
    bounds = np.concatenate([[0], np.cumsum(counts)])
    w13s_i = _interleave_w13(np.asarray(w1s[0], np.float32),
                             np.asarray(w3s[0], np.float32), np_dt)
    w2s_c = np.asarray(w2s[0], np.float32).astype(np_dt)

    in_maps = []
    tok_per_core = []
    for e2 in range(N_CORES):
        lo, hi = int(bounds[e2]), int(bounds[e2 + 1])
        toks = token_idx[lo:hi]
        tok_per_core.append(toks)
        xfull = np.zeros((C + S, dim), np.float32)
        xfull[: hi - lo] = xt[toks] * scores_sorted[lo:hi, None]
        xfull[C:] = xt[e2 * S:(e2 + 1) * S]
        in_maps.append({
            "xT": np.ascontiguousarray(xfull.T).astype(np_dt),
            "w13": _interleave_w13(np.asarray(w1[e2], np.float32),
                                   np.asarray(w3[e2], np.float32), np_dt),
            "w2": np.asarray(w2[e2], np.float32).astype(np_dt),
            "w13s": w13s_i,
            "w2s": w2s_c,
        })

    nc = build_program(C, S)
    if _run is None:
        from concourse.bass_utils import run_bass_kernel_spmd
        results = run_bass_kernel_spmd(nc, in_maps, list(range(N_CORES))).results
    else:
        results = _run(nc, in_maps)

    out = np.empty((N, dim), np.float32)
    for e2 in range(N_CORES):
        y = np.asarray(results[e2]["yT"], dtype=np.float32)
        out[e2 * S:(e2 + 1) * S] = y[:, C:].T
    for e2 in range(N_CORES):
        cnt = len(tok_per_core[e2])
        out[tok_per_core[e2]] += np.asarray(
            results[e2]["yT"][:, :cnt], dtype=np.float32).T
    return out.reshape(bs, slen, dim)



# revision 20
# speedup vs baseline: 133.9994x; 133.9994x over previous
"""MoE (top-2, 8 experts, SwiGLU + shared expert) on 8 TRN2 NeuronCores.

Expert-parallel, bf16 compute.  Host computes the (tiny) router, gathers
each expert's tokens into a padded [C, DIM] block (pre-scaled by router
score), appends the core's 1/8 shared-expert token shard, and ships core e
ONE feature-major activation tensor xT [DIM, C+S] (bf16) plus its expert
weights (w1/w3 column-interleaved w13 [DIM, 2*HIDDEN], w2 [HIDDEN, DIM])
and the replicated shared-expert weights.  The core runs two dense SwiGLU
MLPs feature-major and writes ONE packed output yT [DIM, C+S] (bf16);
host scatter-adds the routed columns into the shared-expert output.

Device schedule (raw Bass, manual semaphores — at most one inline sync
wait / one then_inc per instruction):
  - 34 warm-up matmuls on garbage SBUF (plus gap-fillers between the
    first pass's k-tile waits) give the PE ~3.6 us of sustained
    activity, so the PE_HAM clock gate opens (1.2 -> 2.4 GHz) at
    ~11.5 us instead of ~20 us (measured; the HAM watches a free-
    running 4096-cycle activity window).
  - the head is HBM-bound: pass 0 needs x (1.7 MB) + its w13 k-tiles
    (0.5 MB) and pass 1 another 0.5 MB, so pass 1 cannot start before
    ~14 us at the ~470 GB/s the 16 SDMA engines deliver.  x streams
    per-k-tile on scalar while the first w13 block streams per-k-tile
    on sync (k-tile kk of both must land before the PE's kk-burst).
    Remaining weights stream as 256/512-col blocks through a 4-slot
    ring (one DMA moves all 8 k-tiles; 512-B+ row segments keep the
    SDMA engines at line rate — smaller segments drop to RMW speed).
  - phase order: routed-A, shared-A, routed-B, shared-B.  A-phases
    produce g = silu(h1)*h3 (bf16) per hidden tile; B-phases contract
    g with w2 and stream m-tiles of yT out (shared-B last: its 256-col
    tail chain is the shortest program drain).
  - PSUM double-banking: A/B routed passes use 4-bank sets (2 m-tiles x
    2 token chunks) alternating {0-3}/{4-7}; shared passes use 2-bank
    sets rotating through all 8.
  - (measured dead ends: fp8 DoubleRow would halve PE time but fails
    the 2e-2 gate — 6.5e-2 full-fp8, 3.8e-2 even B-GEMM-only; the
    Block postamble resets a fixed ~51-semaphore range per engine, so
    shrinking the kernel's semaphore count does not shrink it.)

Engine roles:
  sync  (SP) : weight DMAs + shared-B output DMAs
  tensor(PE) : all matmuls (bf16, 1 col/cycle)
  scalar(ACT): x k-tile loads; silu eviction PSUM->SBUF; routed-B
               output DMAs
  vector(DVE): g = silu(h1)*h3 multiply; PSUM->SBUF output copies
"""

from contextlib import ExitStack

import numpy as np

import concourse.bass as bass
import concourse.mybir as mybir

DIM = 1024
HIDDEN = 1024
NUM_EXPERTS = 8
TOP_K = 2
N_CORES = 8
P = 128
KT = DIM // P

DT = mybir.dt.bfloat16
W_SLOTS = 4   # weight-block ring (each slot holds up to [128, 8*512])
S_RING = 4    # silu scratch ring
N_WARMUP = 34  # PE warm-up matmuls: ~3.6 us of sustained PE activity,
               # enough to cover the HAM 4096-cycle activity window so
               # the 1.2->2.4 GHz clock gate opens before real matmuls
ACT_FUNC = mybir.ActivationFunctionType.Silu


def _chunks(total, maxc=512):
    """Split `total` into nearly-equal chunks of at most `maxc` columns."""
    if total <= maxc:
        return [(0, total)]
    n = (total + maxc - 1) // maxc
    h = ((total + n - 1) // n + 15) // 16 * 16
    out, off = [], 0
    while off + h < total:
        out.append((off, h))
        off += h
    out.append((off, total - off))
    return out


class Plan:
    """Per-engine instruction streams with planned semaphore counters."""

    ENGINES = ("sync", "tensor", "scalar", "vector", "gpsimd")

    def __init__(self):
        self.streams = {e: [] for e in self.ENGINES}
        self.cnt = {}
        self._waited = {}

    def wait(self, eng, sem, val):
        val = int(val)
        if val <= 0 or self._waited.get((eng, sem), 0) >= val:
            return
        self._waited[(eng, sem)] = val
        self.streams[eng].append(("wait", sem, val))

    def op(self, eng, fn, incs=()):
        self.streams[eng].append(("op", fn, tuple(incs)))
        for s, v in incs:
            self.cnt[s] = self.cnt.get(s, 0) + v


def build_program(C, S, act_func=ACT_FUNC, repeat=1):
    XW = C + S
    rch = _chunks(C)          # routed token chunks (local cols)
    sch = _chunks(S)          # shared token chunks
    assert len(rch) <= 2 and len(sch) <= 2

    nc = bass.Bass()
    tens = {}
    tens["xT"] = nc.declare_dram_parameter("xT", [DIM, XW], DT, isOutput=False)
    tens["w13"] = nc.declare_dram_parameter("w13", [DIM, 2 * HIDDEN], DT,
                                            isOutput=False)
    tens["w2"] = nc.declare_dram_parameter("w2", [HIDDEN, DIM], DT,
                                           isOutput=False)
    tens["w13s"] = nc.declare_dram_parameter("w13s", [DIM, 2 * HIDDEN], DT,
                                             isOutput=False)
    tens["w2s"] = nc.declare_dram_parameter("w2s", [HIDDEN, DIM], DT,
                                            isOutput=False)
    tens["yT"] = nc.declare_dram_parameter("yT", [DIM, XW], DT, isOutput=True)

    cwmax = max(cw for _, cw in rch + sch)

    # ---- weight block table ----------------------------------------
    # Ring blocks b = 0..11, slot = b % W_SLOTS, sem w{b % W_SLOTS}.
    # (name, m0, mcols); consuming passes are listed per block.  The
    # first routed-A block (pass 0, cols 0-255 of w13) lives in its own
    # wtk tensor, k-split in two DMAs (sems wka / wkb) for a fast start.
    #   passes 0-7   routed A (w13, 256 cols per pass)
    #   passes 8-15  shared A (w13s)
    #   passes 16-23 routed B (w2, 128-col m-tile per pass)
    #   passes 24-31 shared B (w2s)
    blocks = [
        ("w13", 256, 256, [1]),
        ("w13", 512, 512, [2, 3]),
        ("w13", 1024, 512, [4, 5]),
        ("w13", 1536, 512, [6, 7]),
        ("w13s", 0, 512, [8, 9]),
        ("w13s", 512, 512, [10, 11]),
        ("w13s", 1024, 512, [12, 13]),
        ("w13s", 1536, 512, [14, 15]),
        ("w2", 0, 512, [16, 17, 18, 19]),
        ("w2", 512, 512, [20, 21, 22, 23]),
        ("w2s", 0, 512, [24, 25, 26, 27]),
        ("w2s", 512, 512, [28, 29, 30, 31]),
    ]
    # pass -> (block, m-offset in 128-col units within the block)
    pass_block = {}
    for b, (_nm, m0, mc, passes) in enumerate(blocks):
        for i, p in enumerate(passes):
            if p < 16:          # A pass: consumes 256 cols (2 m-tiles)
                pass_block[p] = (b, 2 * i)
            else:               # B pass: consumes 128 cols (1 m-tile)
                pass_block[p] = (b, i)

    plan = Plan()
    st = {"pass_idx": 0, "s_idx": 0,
          "bank_rel": [None] * 8, "s_rel": [None] * S_RING,
          "o_rel": {}, "od_cnt": {}}

    # per-slot release pass: block freed once its last consumer completes
    slot_free = [0] * W_SLOTS

    def weight_dma(b):
        """Issue ring block b ([1024, mc] -> slot [128, KT*mc]) on sync."""
        nm, m0, mc, passes = blocks[b]
        slot = b % W_SLOTS
        plan.wait("sync", "mm", slot_free[slot])

        def fn(e, _slot=slot, _nm=nm, _m0=m0, _mc=mc):
            src = tens[_nm][:, _m0:_m0 + _mc].rearrange(
                "(kk p) c -> p kk c", kk=KT)
            return e.dma_start(out=tens[f"wt{_slot}"][:, :KT * _mc], in_=src)

        wsem = f"w{slot}"
        plan.op("sync", fn, incs=((wsem, 16),))
        slot_free[slot] = max(passes) + 1
        return wsem, plan.cnt[wsem]

    def pe_pass(wtname, mc, moff, wsem_vals, rhs_base, chunks, bankset,
                g_wait=None, first=False, n_ml=2):
        """The 8-k-tile matmul burst of one pass (n_ml m-tiles x chunks).

        wtname/mc/moff: weight tensor, its block width per k-tile, and the
        first 128-col m-sub-tile this pass covers.  wsem_vals: list of
        (sem, val) waits, or per-kk callable when first=True."""
        p = st["pass_idx"]
        if not first:
            for sv in wsem_vals:
                plan.wait("tensor", *sv)
            for kk in range(KT):
                plan.wait("tensor", f"x{kk}", 16)
        ncn = len(chunks)
        n_mm = KT * n_ml * ncn
        i = 0
        for kk in range(KT):
            if first:
                if kk:
                    # garbage matmuls into bank 7 (pass 1 reopens it with
                    # start=True) keep the PE busy across the x-supply
                    # bubbles so the HAM clock gate opens at ~11 us
                    for _ in range(3):
                        def wmm(e):
                            return e.matmul(tens["pb7"][:, :P],
                                            lhsT=tens["wt0"][:, :P],
                                            rhs=tens["wt1"][:, :P],
                                            start=True, stop=True,
                                            skip_group_check=True)

                        plan.op("tensor", wmm)
                plan.wait("tensor", f"x{kk}", 16)
                plan.wait("tensor", f"wk{kk}", 16)
            if g_wait is not None:
                plan.wait("tensor", "g", g_wait(kk))
            for ml in range(n_ml):
                for ci, (c0, cw) in enumerate(chunks):
                    b = bankset[ml * ncn + ci]
                    if kk == 0 and st["bank_rel"][b] is not None:
                        rs, rv = st["bank_rel"][b]
                        plan.wait("tensor", rs, rv)
                    i += 1
                    incs = (("mm", 1),) if i == n_mm else ()

                    def mmop(e, _b=b, _wt=wtname, _kk=kk, _ml=moff + ml,
                             _c0=c0, _cw=cw, _rb=rhs_base, _mc=mc):
                        return e.matmul(
                            tens[f"pb{_b}"][:, :_cw],
                            lhsT=tens[_wt][:, _kk * _mc + _ml * P:
                                           _kk * _mc + (_ml + 1) * P],
                            rhs=tens["xg"][:, _kk * XW + _rb + _c0:
                                           _kk * XW + _rb + _c0 + _cw],
                            start=(_kk == 0), stop=(_kk == KT - 1),
                            skip_group_check=True)

                    plan.op("tensor", mmop, incs=incs)
        st["pass_idx"] += 1

    def a_tail(j, xbase, chunks, bankset):
        """Post-matmul work of A-pass j: silu on ACT, g-mul on DVE."""
        pdone = st["pass_idx"]  # mm value when this pass completes
        ncn = len(chunks)
        for ci, (c0, cw) in enumerate(chunks):
            st["s_idx"] += 1
            s_slot = st["s_idx"] % S_RING
            plan.wait("scalar", "mm", pdone)
            if st["s_rel"][s_slot] is not None:
                rs, rv = st["s_rel"][s_slot]
                plan.wait("scalar", rs, rv)

            def silu(e, _s=s_slot, _b=bankset[ci], _cw=cw):
                return e.activation(tens[f"s{_s}"][:, :_cw],
                                    tens[f"pb{_b}"][:, :_cw], act_func)

            plan.op("scalar", silu, incs=(("s", 1),))
            st["bank_rel"][bankset[ci]] = ("s", plan.cnt["s"])
            s_need = plan.cnt["s"]
            plan.wait("vector", "mm", pdone)
            plan.wait("vector", "s", s_need)
            b3 = bankset[ncn + ci]

            def mul(e, _j=j, _s=s_slot, _b=b3, _xb=xbase, _c0=c0, _cw=cw):
                col = _j * XW + GHALF + _xb + _c0
                return e.tensor_mul(tens["xg"][:, col:col + _cw],
                                    tens[f"s{_s}"][:, :_cw],
                                    tens[f"pb{_b}"][:, :_cw])

            plan.op("vector", mul, incs=(("g", 1),))
            st["bank_rel"][b3] = ("g", plan.cnt["g"])
            st["s_rel"][s_slot] = ("g", plan.cnt["g"])

    def b_tail(m, bankset, chunks, otile, odsem, odeng, ybase,
               pipeline=False):
        """Post-matmul work of B-pass for m-tile m: copy on DVE, output
        DMA on `odeng` with sem `odsem` (one in-flight DMA per sem).
        pipeline=True (single-bank pass only): split the output in two
        halves with per-half copy->DMA pipelining, shortening the
        end-of-program drain chain on the very last pass."""
        pdone = st["pass_idx"]
        plan.wait("vector", "mm", pdone)
        key = (otile, odsem)
        if key in st["o_rel"]:
            plan.wait("vector", odsem, st["o_rel"][key])

        def odma_piece(c0, cw):
            st["od_cnt"][odsem] = st["od_cnt"].get(odsem, 0) + 16

            def odma(e, _o=otile, _m=m, _yb=ybase + c0, _c0=c0, _cw=cw):
                return e.dma_start(
                    out=tens["yT"][_m * P:(_m + 1) * P, _yb:_yb + _cw],
                    in_=tens[f"ot{_o}"][:, _c0:_c0 + _cw])

            plan.op(odeng, odma, incs=((odsem, 16),))

        if not pipeline:
            for ci, (c0, cw) in enumerate(chunks):
                b = bankset[ci]

                def cp(e, _o=otile, _b=b, _c0=c0, _cw=cw):
                    return e.tensor_copy(tens[f"ot{_o}"][:, _c0:_c0 + _cw],
                                         tens[f"pb{_b}"][:, :_cw])

                plan.op("vector", cp, incs=(("o", 1),))
                st["bank_rel"][b] = ("o", plan.cnt["o"])
            span = chunks[-1][0] + chunks[-1][1]
            plan.wait(odeng, "o", plan.cnt["o"])
            odma_piece(0, span)
        else:
            assert len(chunks) == 1
            b = bankset[0]
            total = chunks[0][1]
            h = total // 2
            for c0, cw in ((0, h), (h, total - h)):
                def cp(e, _o=otile, _b=b, _c0=c0, _cw=cw):
                    return e.tensor_copy(tens[f"ot{_o}"][:, _c0:_c0 + _cw],
                                         tens[f"pb{_b}"][:, _c0:_c0 + _cw])

                plan.op("vector", cp, incs=(("o", 1),))
                plan.wait(odeng, "o", plan.cnt["o"])
                odma_piece(c0, cw)
            st["bank_rel"][b] = ("o", plan.cnt["o"])
        st["o_rel"][key] = st["od_cnt"][odsem]

    with ExitStack() as ctx:
        def sb(name, shape, dt):
            tens[name] = ctx.enter_context(nc.sbuf_tensor(name, shape, dt))

        # xg holds x tiles (first KT*XW cols) and g tiles (next KT*XW)
        GHALF = KT * XW
        sb("xg", [P, 2 * KT * XW], DT)
        sb("wtk", [P, KT * 256], DT)
        for r in range(W_SLOTS):
            sb(f"wt{r}", [P, KT * 512], DT)
        for r in range(S_RING):
            sb(f"s{r}", [P, cwmax], DT)
        for r in range(4):
            sb(f"ot{r}", [P, C], DT)
        for r in range(4, 8):
            sb(f"ot{r}", [P, S], DT)
        for b in range(8):
            tens[f"pb{b}"] = ctx.enter_context(
                nc.psum_tensor(f"pb{b}", [P, 512], mybir.dt.float32))

        r4 = [(0, 1, 2, 3), (4, 5, 6, 7)]       # routed 4-bank sets
        s2 = [(0, 1), (2, 3), (4, 5), (6, 7)]   # shared 2-bank sets

        # ---- PE warm-up: garbage matmuls, results discarded ----------
        for _ in range(N_WARMUP):
            def wmm(e):
                return e.matmul(tens["pb7"][:, :P],
                                lhsT=tens["wt0"][:, :P],
                                rhs=tens["wt1"][:, :P],
                                start=True, stop=True,
                                skip_group_check=True)

            plan.op("tensor", wmm)

        for _rep in range(repeat):
            g0 = plan.cnt.get("g", 0)

            # -- x loads: all on the scalar HWDGE queue ----------------
            for kk in range(KT):
                def xl(e, _kk=kk):
                    return e.dma_start(
                        out=tens["xg"][:, _kk * XW:(_kk + 1) * XW],
                        in_=tens["xT"][_kk * P:(_kk + 1) * P, :])

                plan.op("scalar", xl, incs=((f"x{kk}", 16),))

            # -- first w13 block (cols 0-255): per-k-tile on sync so the
            # -- PE can start on k-tile 0 ~0.7 us after issue; the pass-1
            # -- block (b0) is prefetched right after k-tile 0 ----------
            wv = {}
            for kk in range(KT):
                def wl(e, _kk=kk):
                    return e.dma_start(
                        out=tens["wtk"][:, _kk * 256:(_kk + 1) * 256],
                        in_=tens["w13"][_kk * P:(_kk + 1) * P, 0:256])

                plan.op("sync", wl, incs=((f"wk{kk}", 16),))

            # -- prefetch ring blocks as slots free (after the wk tiles:
            # -- the head is HBM-bandwidth-bound, so blocks are ordered
            # -- strictly by first-need time) --------------------------
            for b in range(len(blocks)):
                wv[b] = weight_dma(b)

            # -- routed A: 8 passes ------------------------------------
            pe_pass("wtk", 256, 0, None, 0, rch, r4[0], first=True)
            a_tail(0, 0, rch, r4[0])
            for j in range(1, KT):
                b, moff = pass_block[j]
                pe_pass(f"wt{b % W_SLOTS}", blocks[b][2], moff, [wv[b]],
                        0, rch, r4[j % 2])
                a_tail(j, 0, rch, r4[j % 2])
            g_routed_end = plan.cnt.get("g", 0)

            # -- shared A: 8 passes ------------------------------------
            for j in range(KT):
                p = 8 + j
                b, moff = pass_block[p]
                pe_pass(f"wt{b % W_SLOTS}", blocks[b][2], moff, [wv[b]],
                        C, sch, s2[j % 4])
                a_tail(j, C, sch, s2[j % 4])

            # -- routed B: 8 m-passes, output DMAs on scalar -----------
            for m in range(KT):
                p = 16 + m
                b, moff = pass_block[p]
                pe_pass(f"wt{b % W_SLOTS}", blocks[b][2], moff, [wv[b]],
                        GHALF, rch, s2[m % 4],
                        g_wait=lambda kk, _g0=g0: _g0 + len(rch) * (kk + 1),
                        n_ml=1)
                b_tail(m, s2[m % 4], rch, m % 4, f"odr{m % 4}", "scalar", 0)

            # -- shared B: 8 m-passes, output DMAs on sync -------------
            for m in range(KT):
                p = 24 + m
                b, moff = pass_block[p]
                pe_pass(f"wt{b % W_SLOTS}", blocks[b][2], moff, [wv[b]],
                        GHALF + C, sch, (m % 8,),
                        g_wait=lambda kk, _g0=g_routed_end: _g0 + kk + 1,
                        n_ml=1)
                b_tail(m, (m % 8,), sch, 4 + m % 4, f"ods{m % 4}", "sync", C)

        # ---- drain: wait for the last output DMAs on their queues ----
        for i in range(4):
            if plan.cnt.get(f"odr{i}", 0):
                plan.wait("scalar", f"odr{i}", plan.cnt[f"odr{i}"])
            if plan.cnt.get(f"ods{i}", 0):
                plan.wait("sync", f"ods{i}", plan.cnt[f"ods{i}"])

        # ---- emit ----
        with ExitStack() as sem_ctx:
            sems = {}
            for name in plan.cnt:
                sems[name] = sem_ctx.enter_context(nc.semaphore(f"sem_{name}"))

            with nc.Block() as block:
                def runner(stream):
                    def run(e):
                        for item in stream:
                            if item[0] == "wait":
                                _, s, v = item
                                e.wait_ge(sems[s], v)
                            else:
                                _, fn, incs = item
                                inst = fn(e)
                                rest = list(incs)
                                if rest and inst is not None:
                                    s, v = rest.pop(0)
                                    inst.then_inc(sems[s], v)
                                for s, v in rest:
                                    e.sem_inc(sems[s], v)
                    return run

                block.sync(runner(plan.streams["sync"]))
                block.tensor(runner(plan.streams["tensor"]))
                block.scalar(runner(plan.streams["scalar"]))
                block.vector(runner(plan.streams["vector"]))
                if plan.streams["gpsimd"]:
                    block.gpsimd(runner(plan.streams["gpsimd"]))
    return nc


def _interleave_w13(w1e, w3e, np_dt):
    out = np.empty((DIM, 2 * HIDDEN), dtype=np.float32)
    v = out.reshape(DIM, HIDDEN // P, 2, P)
    v[:, :, 0, :] = w1e.reshape(DIM, HIDDEN // P, P)
    v[:, :, 1, :] = w3e.reshape(DIM, HIDDEN // P, P)
    return out.astype(np_dt)


def route(xt, gate_w):
    logits = (xt @ gate_w.T).astype(np.float32)
    m = logits.max(axis=1, keepdims=True)
    e = np.exp(logits - m)
    scores = (e / e.sum(axis=1, keepdims=True)).astype(np.float32)
    sel = np.argsort(-scores, axis=1, kind="stable")[:, :TOP_K].astype(np.int32)
    top_scores = np.take_along_axis(scores, sel, axis=1)
    sel_flat = sel.reshape(-1)
    order = np.argsort(sel_flat, kind="stable")
    token_idx = (order // TOP_K).astype(np.int64)
    eid = sel_flat[order]
    scores_sorted = top_scores.reshape(-1)[order]
    return token_idx, eid, scores_sorted


def kernel(x, gate_w, w1, w2, w3, w1s, w2s, w3s, _run=None):
    x = np.asarray(x, dtype=np.float32)
    bs, slen, dim = x.shape
    N = bs * slen
    xt = np.ascontiguousarray(x.reshape(N, dim))
    S = N // N_CORES

    token_idx, eid, scores_sorted = route(xt, np.asarray(gate_w, np.float32))

    counts = np.bincount(eid, minlength=NUM_EXPERTS)
    C = int(max(256, ((counts.max() + 7) // 8) * 8))

    np_dt = mybir.dt.np(DT)
    bounds = np.concatenate([[0], np.cumsum(counts)])
    w13s_i = _interleave_w13(np.asarray(w1s[0], np.float32),
                             np.asarray(w3s[0], np.float32), np_dt)
    w2s_c = np.asarray(w2s[0], np.float32).astype(np_dt)

    in_maps = []
    tok_per_core = []
    for e2 in range(N_CORES):
        lo, hi = int(bounds[e2]), int(bounds[e2 + 1])
        toks = token_idx[lo:hi]
        tok_per_core.append(toks)
        xfull = np.zeros((C + S, dim), np.float32)
        xfull[: hi - lo] = xt[toks] * scores_sorted[lo:hi, None]
        xfull[C:] = xt[e2 * S:(e2 + 1) * S]
        in_maps.append({
            "xT": np.ascontiguousarray(xfull.T).astype(np_dt),
            "w13": _interleave_w13(np.asarray(w1[e2], np.float32),
                                   np.asarray(w3[e2], np.float32), np_dt),
            "w2": np.asarray(w2[e2], np.float32).astype(np_dt),
            "w13s": w13s_i,
            "w2s": w2s_c,
        })

    nc = build_program(C, S)
    if _run is None:
        from concourse.bass_utils import run_bass_kernel_spmd
        results = run_bass_kernel_spmd(nc, in_maps, list(range(N_CORES))).results
    else:
        results = _run(nc, in_maps)

    out = np.empty((N, dim), np.float32)
    for e2 in range(N_CORES):
        y = np.asarray(results[e2]["yT"], dtype=np.float32)
        out[e2 * S:(e2 + 1) * S] = y[:, C:].T
    for e2 in range(N_CORES):
        cnt = len(tok_per_core[e2])
        out[tok_per_core[e2]] += np.asarray(
            results[e2]["yT"][:, :cnt], dtype=np.float32).T
    return out.reshape(bs, slen, dim)


# revision 27
# speedup vs baseline: 136.1970x; 1.0164x over previous
"""MoE (top-2, 8 experts, SwiGLU + shared expert) on 8 TRN2 NeuronCores.

Expert-parallel, bf16 compute.  Host computes the (tiny) router, gathers
each expert's tokens into a padded [C, DIM] block (pre-scaled by router
score), appends the core's 1/8 shared-expert token shard, and ships core e
ONE feature-major activation tensor xT [DIM, C+S] (bf16) plus its expert
weights (w1/w3 column-interleaved w13 [DIM, 2*HIDDEN], w2 [HIDDEN, DIM])
and the replicated shared-expert weights.  The core runs two dense SwiGLU
MLPs feature-major and writes ONE packed output yT [DIM, C+S] (bf16);
host scatter-adds the routed columns into the shared-expert output.

Device schedule (raw Bass, manual semaphores — at most one inline sync
wait / one then_inc per instruction):
  - 34 warm-up matmuls on garbage SBUF (plus gap-fillers between the
    first pass's k-tile waits) give the PE ~3.6 us of sustained
    activity, so the PE_HAM clock gate opens (1.2 -> 2.4 GHz) at
    ~11.5 us instead of ~20 us (measured; the HAM watches a free-
    running 4096-cycle activity window).
  - the head is HBM-bound: pass 0 needs x (1.7 MB) + its w13 k-tiles
    (0.5 MB) and pass 1 another 0.5 MB, so pass 1 cannot start before
    ~14 us at the ~470 GB/s the 16 SDMA engines deliver.  x streams
    per-k-tile on scalar while the first w13 block streams per-k-tile
    on sync (k-tile kk of both must land before the PE's kk-burst).
    Remaining weights stream as 256/512-col blocks through a 4-slot
    ring (one DMA moves all 8 k-tiles; 512-B+ row segments keep the
    SDMA engines at line rate — smaller segments drop to RMW speed).
  - phase order: routed-A, shared-A, routed-B, shared-B.  A-phases
    produce g = silu(h1)*h3 (bf16) per hidden tile; B-phases contract
    g with w2 and stream m-tiles of yT out (shared-B last: its 256-col
    tail chain is the shortest program drain).
  - PSUM double-banking: A/B routed passes use 4-bank sets (2 m-tiles x
    2 token chunks) alternating {0-3}/{4-7}; shared passes use 2-bank
    sets rotating through all 8.
  - (measured dead ends: fp8 DoubleRow would halve PE time but fails
    the 2e-2 gate — 6.5e-2 full-fp8, 3.8e-2 even B-GEMM-only; the
    Block postamble resets a fixed ~51-semaphore range per engine, so
    shrinking the kernel's semaphore count does not shrink it.)

Engine roles:
  sync  (SP) : weight DMAs + shared-B output DMAs
  tensor(PE) : all matmuls (bf16, 1 col/cycle)
  scalar(ACT): x k-tile loads; silu eviction PSUM->SBUF; routed-B
               output DMAs
  vector(DVE): g = silu(h1)*h3 multiply; PSUM->SBUF output copies
"""

from contextlib import ExitStack

import numpy as np

import concourse.bass as bass
import concourse.mybir as mybir

DIM = 1024
HIDDEN = 1024
NUM_EXPERTS = 8
TOP_K = 2
N_CORES = 8
P = 128
KT = DIM // P

DT = mybir.dt.bfloat16
W_SLOTS = 4   # weight-block ring (each slot holds up to [128, 8*512])
S_RING = 4    # silu scratch ring
N_WARMUP = 34  # PE warm-up matmuls: ~3.6 us of sustained PE activity,
               # enough to cover the HAM 4096-cycle activity window so
               # the 1.2->2.4 GHz clock gate opens before real matmuls
ACT_FUNC = mybir.ActivationFunctionType.Silu


def _chunks(total, maxc=512):
    """Split `total` into nearly-equal chunks of at most `maxc` columns."""
    if total <= maxc:
        return [(0, total)]
    n = (total + maxc - 1) // maxc
    h = ((total + n - 1) // n + 15) // 16 * 16
    out, off = [], 0
    while off + h < total:
        out.append((off, h))
        off += h
    out.append((off, total - off))
    return out


class Plan:
    """Per-engine instruction streams with planned semaphore counters."""

    ENGINES = ("sync", "tensor", "scalar", "vector", "gpsimd")

    def __init__(self):
        self.streams = {e: [] for e in self.ENGINES}
        self.cnt = {}
        self._waited = {}

    def wait(self, eng, sem, val):
        val = int(val)
        if val <= 0 or self._waited.get((eng, sem), 0) >= val:
            return
        self._waited[(eng, sem)] = val
        self.streams[eng].append(("wait", sem, val))

    def op(self, eng, fn, incs=()):
        self.streams[eng].append(("op", fn, tuple(incs)))
        for s, v in incs:
            self.cnt[s] = self.cnt.get(s, 0) + v


def build_program(C, S, act_func=ACT_FUNC, repeat=1):
    XW = C + S
    rch = _chunks(C)          # routed token chunks (local cols)
    sch = _chunks(S)          # shared token chunks
    assert len(rch) <= 2 and len(sch) <= 2

    nc = bass.Bass()
    tens = {}
    tens["xT"] = nc.declare_dram_parameter("xT", [DIM, XW], DT, isOutput=False)
    tens["w13"] = nc.declare_dram_parameter("w13", [DIM, 2 * HIDDEN], DT,
                                            isOutput=False)
    tens["w2"] = nc.declare_dram_parameter("w2", [HIDDEN, DIM], DT,
                                           isOutput=False)
    tens["w13s"] = nc.declare_dram_parameter("w13s", [DIM, 2 * HIDDEN], DT,
                                             isOutput=False)
    tens["w2s"] = nc.declare_dram_parameter("w2s", [HIDDEN, DIM], DT,
                                            isOutput=False)
    tens["yT"] = nc.declare_dram_parameter("yT", [DIM, XW], DT, isOutput=True)

    cwmax = max(cw for _, cw in rch + sch)

    # ---- weight block table ----------------------------------------
    # Ring blocks b = 0..11, slot = b % W_SLOTS, sem w{b % W_SLOTS}.
    # (name, m0, mcols); consuming passes are listed per block.  The
    # first routed-A block (pass 0, cols 0-255 of w13) lives in its own
    # wtk tensor, k-split in two DMAs (sems wka / wkb) for a fast start.
    #   passes 0-7   routed A (w13, 256 cols per pass)
    #   passes 8-15  shared A (w13s)
    #   passes 16-23 routed B (w2, 128-col m-tile per pass)
    #   passes 24-31 shared B (w2s)
    blocks = [
        ("w13", 256, 256, [1]),
        ("w13", 512, 512, [2, 3]),
        ("w13", 1024, 512, [4, 5]),
        ("w13", 1536, 512, [6, 7]),
        ("w13s", 0, 512, [8, 9]),
        ("w13s", 512, 512, [10, 11]),
        ("w13s", 1024, 512, [12, 13]),
        ("w13s", 1536, 512, [14, 15]),
        ("w2", 0, 512, [16, 17, 18, 19]),
        ("w2", 512, 512, [20, 21, 22, 23]),
        ("w2s", 0, 512, [24, 25, 26, 27]),
        ("w2s", 512, 512, [28, 29, 30, 31]),
    ]
    # pass -> (block, m-offset in 128-col units within the block)
    pass_block = {}
    for b, (_nm, m0, mc, passes) in enumerate(blocks):
        for i, p in enumerate(passes):
            if p < 16:          # A pass: consumes 256 cols (2 m-tiles)
                pass_block[p] = (b, 2 * i)
            else:               # B pass: consumes 128 cols (1 m-tile)
                pass_block[p] = (b, i)

    plan = Plan()
    st = {"pass_idx": 0, "s_idx": 0,
          "bank_rel": [None] * 8, "s_rel": [None] * S_RING,
          "o_rel": {}, "od_cnt": {}}

    # per-slot release pass: block freed once its last consumer completes
    slot_free = [0] * W_SLOTS

    def weight_dma(b):
        """Issue ring block b ([1024, mc] -> slot [128, KT*mc]) on sync."""
        nm, m0, mc, passes = blocks[b]
        slot = b % W_SLOTS
        plan.wait("sync", "mm", slot_free[slot])

        def fn(e, _slot=slot, _nm=nm, _m0=m0, _mc=mc):
            src = tens[_nm][:, _m0:_m0 + _mc].rearrange(
                "(kk p) c -> p kk c", kk=KT)
            return e.dma_start(out=tens[f"wt{_slot}"][:, :KT * _mc], in_=src)

        wsem = f"w{slot}"
        plan.op("sync", fn, incs=((wsem, 16),))
        slot_free[slot] = max(passes) + 1
        return wsem, plan.cnt[wsem]

    def pe_pass(wtname, mc, moff, wsem_vals, rhs_base, chunks, bankset,
                g_wait=None, first=False, n_ml=2):
        """The 8-k-tile matmul burst of one pass (n_ml m-tiles x chunks).

        wtname/mc/moff: weight tensor, its block width per k-tile, and the
        first 128-col m-sub-tile this pass covers.  wsem_vals: list of
        (sem, val) waits, or per-kk callable when first=True."""
        p = st["pass_idx"]
        if not first:
            for sv in wsem_vals:
                plan.wait("tensor", *sv)
            for kk in range(KT):
                plan.wait("tensor", f"x{kk}", 16)
        ncn = len(chunks)
        n_mm = KT * n_ml * ncn
        i = 0
        for kk in range(KT):
            if first:
                if kk:
                    # garbage matmuls into bank 7 (pass 1 reopens it with
                    # start=True) keep the PE busy across the x-supply
                    # bubbles so the HAM clock gate opens at ~11 us
                    for _ in range(3):
                        def wmm(e):
                            return e.matmul(tens["pb7"][:, :P],
                                            lhsT=tens["wt0"][:, :P],
                                            rhs=tens["wt1"][:, :P],
                                            start=True, stop=True,
                                            skip_group_check=True)

                        plan.op("tensor", wmm)
                plan.wait("tensor", f"x{kk}", 16)
                plan.wait("tensor", f"wk{kk}", 16)
            if g_wait is not None:
                plan.wait("tensor", "g", g_wait(kk))
            for ml in range(n_ml):
                for ci, (c0, cw) in enumerate(chunks):
                    b = bankset[ml * ncn + ci]
                    if kk == 0 and st["bank_rel"][b] is not None:
                        rs, rv = st["bank_rel"][b]
                        plan.wait("tensor", rs, rv)
                    i += 1
                    incs = (("mm", 1),) if i == n_mm else ()

                    def mmop(e, _b=b, _wt=wtname, _kk=kk, _ml=moff + ml,
                             _c0=c0, _cw=cw, _rb=rhs_base, _mc=mc):
                        return e.matmul(
                            tens[f"pb{_b}"][:, :_cw],
                            lhsT=tens[_wt][:, _kk * _mc + _ml * P:
                                           _kk * _mc + (_ml + 1) * P],
                            rhs=tens["xg"][:, _kk * XW + _rb + _c0:
                                           _kk * XW + _rb + _c0 + _cw],
                            start=(_kk == 0), stop=(_kk == KT - 1),
                            skip_group_check=True)

                    plan.op("tensor", mmop, incs=incs)
        st["pass_idx"] += 1

    def a_tail(j, xbase, chunks, bankset):
        """Post-matmul work of A-pass j: silu on ACT, g-mul on DVE."""
        pdone = st["pass_idx"]  # mm value when this pass completes
        ncn = len(chunks)
        for ci, (c0, cw) in enumerate(chunks):
            st["s_idx"] += 1
            s_slot = st["s_idx"] % S_RING
            plan.wait("scalar", "mm", pdone)
            if st["s_rel"][s_slot] is not None:
                rs, rv = st["s_rel"][s_slot]
                plan.wait("scalar", rs, rv)

            def silu(e, _s=s_slot, _b=bankset[ci], _cw=cw):
                return e.activation(tens[f"s{_s}"][:, :_cw],
                                    tens[f"pb{_b}"][:, :_cw], act_func)

            plan.op("scalar", silu, incs=(("s", 1),))
            st["bank_rel"][bankset[ci]] = ("s", plan.cnt["s"])
            s_need = plan.cnt["s"]
            plan.wait("vector", "mm", pdone)
            plan.wait("vector", "s", s_need)
            b3 = bankset[ncn + ci]

            def mul(e, _j=j, _s=s_slot, _b=b3, _xb=xbase, _c0=c0, _cw=cw):
                col = _j * XW + GHALF + _xb + _c0
                return e.tensor_mul(tens["xg"][:, col:col + _cw],
                                    tens[f"s{_s}"][:, :_cw],
                                    tens[f"pb{_b}"][:, :_cw])

            plan.op("vector", mul, incs=(("g", 1),))
            st["bank_rel"][b3] = ("g", plan.cnt["g"])
            st["s_rel"][s_slot] = ("g", plan.cnt["g"])

    def b_tail(m, bankset, chunks, otile, odsem, odeng, ybase,
               pipeline=False):
        """Post-matmul work of B-pass for m-tile m: copy on DVE, output
        DMA on `odeng` with sem `odsem` (one in-flight DMA per sem).
        pipeline=True (single-bank pass only): split the output in two
        halves with per-half copy->DMA pipelining, shortening the
        end-of-program drain chain on the very last pass."""
        pdone = st["pass_idx"]
        plan.wait("vector", "mm", pdone)
        key = (otile, odsem)
        if key in st["o_rel"]:
            plan.wait("vector", odsem, st["o_rel"][key])

        def odma_piece(c0, cw):
            st["od_cnt"][odsem] = st["od_cnt"].get(odsem, 0) + 16

            def odma(e, _o=otile, _m=m, _yb=ybase + c0, _c0=c0, _cw=cw):
                return e.dma_start(
                    out=tens["yT"][_m * P:(_m + 1) * P, _yb:_yb + _cw],
                    in_=tens[f"ot{_o}"][:, _c0:_c0 + _cw])

            plan.op(odeng, odma, incs=((odsem, 16),))

        if not pipeline:
            for ci, (c0, cw) in enumerate(chunks):
                b = bankset[ci]

                def cp(e, _o=otile, _b=b, _c0=c0, _cw=cw):
                    return e.tensor_copy(tens[f"ot{_o}"][:, _c0:_c0 + _cw],
                                         tens[f"pb{_b}"][:, :_cw])

                plan.op("vector", cp, incs=(("o", 1),))
                st["bank_rel"][b] = ("o", plan.cnt["o"])
            span = chunks[-1][0] + chunks[-1][1]
            plan.wait(odeng, "o", plan.cnt["o"])
            odma_piece(0, span)
        else:
            assert len(chunks) == 1
            b = bankset[0]
            total = chunks[0][1]
            h = total // 2
            for c0, cw in ((0, h), (h, total - h)):
                def cp(e, _o=otile, _b=b, _c0=c0, _cw=cw):
                    return e.tensor_copy(tens[f"ot{_o}"][:, _c0:_c0 + _cw],
                                         tens[f"pb{_b}"][:, _c0:_c0 + _cw])

                plan.op("vector", cp, incs=(("o", 1),))
                plan.wait(odeng, "o", plan.cnt["o"])
                odma_piece(c0, cw)
            st["bank_rel"][b] = ("o", plan.cnt["o"])
        st["o_rel"][key] = st["od_cnt"][odsem]

    with ExitStack() as ctx:
        def sb(name, shape, dt):
            tens[name] = ctx.enter_context(nc.sbuf_tensor(name, shape, dt))

        # xg holds x tiles (first KT*XW cols) and g tiles (next KT*XW)
        GHALF = KT * XW
        sb("xg", [P, 2 * KT * XW], DT)
        sb("wtk", [P, KT * 256], DT)
        for r in range(W_SLOTS):
            sb(f"wt{r}", [P, KT * 512], DT)
        for r in range(S_RING):
            sb(f"s{r}", [P, cwmax], DT)
        for r in range(4):
            sb(f"ot{r}", [P, C], DT)
        for r in range(4, 8):
            sb(f"ot{r}", [P, S], DT)
        for b in range(8):
            tens[f"pb{b}"] = ctx.enter_context(
                nc.psum_tensor(f"pb{b}", [P, 512], mybir.dt.float32))

        r4 = [(0, 1, 2, 3), (4, 5, 6, 7)]       # routed 4-bank sets
        s2 = [(0, 1), (2, 3), (4, 5), (6, 7)]   # shared 2-bank sets

        # ---- PE warm-up: garbage matmuls, results discarded ----------
        for _ in range(N_WARMUP):
            def wmm(e):
                return e.matmul(tens["pb7"][:, :P],
                                lhsT=tens["wt0"][:, :P],
                                rhs=tens["wt1"][:, :P],
                                start=True, stop=True,
                                skip_group_check=True)

            plan.op("tensor", wmm)

        for _rep in range(repeat):
            g0 = plan.cnt.get("g", 0)

            # -- x loads: all on the scalar HWDGE queue ----------------
            for kk in range(KT):
                def xl(e, _kk=kk):
                    return e.dma_start(
                        out=tens["xg"][:, _kk * XW:(_kk + 1) * XW],
                        in_=tens["xT"][_kk * P:(_kk + 1) * P, :])

                plan.op("scalar", xl, incs=((f"x{kk}", 16),))

            # -- first w13 block (cols 0-255): per-k-tile on sync so the
            # -- PE can start on k-tile 0 ~0.7 us after issue; the pass-1
            # -- block (b0) is prefetched right after k-tile 0 ----------
            wv = {}
            for kk in range(KT):
                def wl(e, _kk=kk):
                    return e.dma_start(
                        out=tens["wtk"][:, _kk * 256:(_kk + 1) * 256],
                        in_=tens["w13"][_kk * P:(_kk + 1) * P, 0:256])

                plan.op("sync", wl, incs=((f"wk{kk}", 16),))

            # -- prefetch ring blocks as slots free (after the wk tiles:
            # -- the head is HBM-bandwidth-bound, so blocks are ordered
            # -- strictly by first-need time) --------------------------
            for b in range(len(blocks)):
                wv[b] = weight_dma(b)

            # -- routed A: 8 passes ------------------------------------
            pe_pass("wtk", 256, 0, None, 0, rch, r4[0], first=True)
            a_tail(0, 0, rch, r4[0])
            for j in range(1, KT):
                b, moff = pass_block[j]
                pe_pass(f"wt{b % W_SLOTS}", blocks[b][2], moff, [wv[b]],
                        0, rch, r4[j % 2])
                a_tail(j, 0, rch, r4[j % 2])
            g_routed_end = plan.cnt.get("g", 0)

            # -- shared A: 8 passes ------------------------------------
            for j in range(KT):
                p = 8 + j
                b, moff = pass_block[p]
                pe_pass(f"wt{b % W_SLOTS}", blocks[b][2], moff, [wv[b]],
                        C, sch, s2[j % 4])
                a_tail(j, C, sch, s2[j % 4])

            # -- routed B: 8 m-passes, output DMAs on scalar -----------
            for m in range(KT):
                p = 16 + m
                b, moff = pass_block[p]
                pe_pass(f"wt{b % W_SLOTS}", blocks[b][2], moff, [wv[b]],
                        GHALF, rch, s2[m % 4],
                        g_wait=lambda kk, _g0=g0: _g0 + len(rch) * (kk + 1),
                        n_ml=1)
                b_tail(m, s2[m % 4], rch, m % 4, f"odr{m % 4}", "scalar", 0)

            # -- shared B: 8 m-passes, output DMAs on sync -------------
            for m in range(KT):
                p = 24 + m
                b, moff = pass_block[p]
                pe_pass(f"wt{b % W_SLOTS}", blocks[b][2], moff, [wv[b]],
                        GHALF + C, sch, (m % 8,),
                        g_wait=lambda kk, _g0=g_routed_end: _g0 + kk + 1,
                        n_ml=1)
                b_tail(m, (m % 8,), sch, 4 + m % 4, f"ods{m % 4}", "sync", C)

        # ---- drain: wait for the last output DMAs on their queues ----
        for i in range(4):
            if plan.cnt.get(f"odr{i}", 0):
                plan.wait("scalar", f"odr{i}", plan.cnt[f"odr{i}"])
            if plan.cnt.get(f"ods{i}", 0):
                plan.wait("sync", f"ods{i}", plan.cnt[f"ods{i}"])

        # ---- emit ----
        with ExitStack() as sem_ctx:
            sems = {}
            for name in plan.cnt:
                sems[name] = sem_ctx.enter_context(nc.semaphore(f"sem_{name}"))

            with nc.Block() as block:
                def runner(stream):
                    def run(e):
                        for item in stream:
                            if item[0] == "wait":
                                _, s, v = item
                                e.wait_ge(sems[s], v)
                            else:
                                _, fn, incs = item
                                inst = fn(e)
                                rest = list(incs)
                                if rest and inst is not None:
                                    s, v = rest.pop(0)
                                    inst.then_inc(sems[s], v)
                                for s, v in rest:
                                    e.sem_inc(sems[s], v)
                    return run

                block.sync(runner(plan.streams["sync"]))
                block.tensor(runner(plan.streams["tensor"]))
                block.scalar(runner(plan.streams["scalar"]))
                block.vector(runner(plan.streams["vector"]))
                if plan.streams["gpsimd"]:
                    block.gpsimd(runner(plan.streams["gpsimd"]))
    return nc


def _interleave_w13(w1e, w3e, np_dt):
    out = np.empty((DIM, 2 * HIDDEN), dtype=np.float32)
    v = out.reshape(DIM, HIDDEN // P, 2, P)
    v[:, :, 0, :] = w1e.reshape(DIM, HIDDEN // P, P)
    v[:, :, 1, :] = w3e.reshape(DIM, HIDDEN // P, P)
    return out.astype(np_dt)


def route(xt, gate_w):
    logits = (xt @ gate_w.T).astype(np.float32)
    m = logits.max(axis=1, keepdims=True)
    e = np.exp(logits - m)
    scores = (e / e.sum(axis=1, keepdims=True)).astype(np.float32)
    sel = np.argsort(-scores, axis=1, kind="stable")[:, :TOP_K].astype(np.int32)
    top_scores = np.take_along_axis(scores, sel, axis=1)
    sel_flat = sel.reshape(-1)
    order = np.argsort(sel_flat, kind="stable")
    token_idx = (order // TOP_K).astype(np.int64)
    eid = sel_flat[order]
    scores_sorted = top_scores.reshape(-1)[order]
    return token_idx, eid, scores_sorted


def kernel(x, gate_w, w1, w2, w3, w1s, w2s, w3s, _run=None):
    x = np.asarray(x, dtype=np.float32)
    bs, slen, dim = x.shape
    N = bs * slen
    xt = np.ascontiguousarray(x.reshape(N, dim))
    S = N // N_CORES

    token_idx, eid, scores_sorted = route(xt, np.asarray(gate_w, np.float32))

    counts = np.bincount(eid, minlength=NUM_EXPERTS)
    C = int(max(256, ((counts.max() + 7) // 8) * 8))

    np_dt = mybir.dt.np(DT)
    bounds = np.concatenate([[0], np.cumsum(counts)])
    w13s_i = _interleave_w13(np.asarray(w1s[0], np.float32),
                             np.asarray(w3s[0], np.float32), np_dt)
    w2s_c = np.asarray(w2s[0], np.float32).astype(np_dt)

    in_maps = []
    tok_per_core = []
    for e2 in range(N_CORES):
        lo, hi = int(bounds[e2]), int(bounds[e2 + 1])
        toks = token_idx[lo:hi]
        tok_per_core.append(toks)
        xfull = np.zeros((C + S, dim), np.float32)
        xfull[: hi - lo] = xt[toks] * scores_sorted[lo:hi, None]
        xfull[C:] = xt[e2 * S:(e2 + 1) * S]
        in_maps.append({
            "xT": np.ascontiguousarray(xfull.T).astype(np_dt),
            "w13": _interleave_w13(np.asarray(w1[e2], np.float32),
                                   np.asarray(w3[e2], np.float32), np_dt),
            "w2": np.asarray(w2[e2], np.float32).astype(np_dt),
            "w13s": w13s_i,
            "w2s": w2s_c,
        })

    nc = build_program(C, S)
    if _run is None:
        from concourse.bass_utils import run_bass_kernel_spmd
        results = run_bass_kernel_spmd(nc, in_maps, list(range(N_CORES))).results
    else:
        results = _run(nc, in_maps)

    out = np.empty((N, dim), np.float32)
    for e2 in range(N_CORES):
        y = np.asarray(results[e2]["yT"], dtype=np.float32)
        out[e2 * S:(e2 + 1) * S] = y[:, C:].T
    for e2 in range(N_CORES):
        cnt = len(tok_per_core[e2])
        out[tok_per_core[e2]] += np.asarray(
            results[e2]["yT"][:, :cnt], dtype=np.float32).T
    return out.reshape(bs, slen, dim)


# revision 31
# speedup vs baseline: 136.3405x; 1.0011x over previous
"""MoE (top-2, 8 experts, SwiGLU + shared expert) on 8 TRN2 NeuronCores.

Expert-parallel, bf16 compute.  Host computes the (tiny) router, gathers
each expert's tokens into a padded [C, DIM] block (pre-scaled by router
score), appends the core's 1/8 shared-expert token shard, and ships core e
ONE feature-major activation tensor xT [DIM, C+S] (bf16) plus its expert
weights (w1/w3 column-interleaved w13 [DIM, 2*HIDDEN], w2 [HIDDEN, DIM])
and the replicated shared-expert weights.  The core runs two dense SwiGLU
MLPs feature-major and writes ONE packed output yT [DIM, C+S] (bf16);
host scatter-adds the routed columns into the shared-expert output.

Device schedule (raw Bass, manual semaphores — at most one inline sync
wait / one then_inc per instruction):
  - 34 warm-up matmuls on garbage SBUF (plus gap-fillers between the
    first pass's k-tile waits) give the PE ~3.6 us of sustained
    activity, so the PE_HAM clock gate opens (1.2 -> 2.4 GHz) at
    ~11.5 us instead of ~20 us (measured; the HAM watches a free-
    running 4096-cycle activity window).
  - the head is HBM-bound: pass 0 needs x (1.7 MB) + its w13 k-tiles
    (0.5 MB) and pass 1 another 0.5 MB, so pass 1 cannot start before
    ~14 us at the ~470 GB/s the 16 SDMA engines deliver.  x streams
    per-k-tile on scalar while the first w13 block streams per-k-tile
    on sync (k-tile kk of both must land before the PE's kk-burst).
    Remaining weights stream as 256/512-col blocks through a 4-slot
    ring (one DMA moves all 8 k-tiles; 512-B+ row segments keep the
    SDMA engines at line rate — smaller segments drop to RMW speed).
  - phase order: routed-A, shared-A, routed-B, shared-B.  A-phases
    produce g = silu(h1)*h3 (bf16) per hidden tile; B-phases contract
    g with w2 and stream m-tiles of yT out (shared-B last: its 256-col
    tail chain is the shortest program drain).
  - PSUM double-banking: A/B routed passes use 4-bank sets (2 m-tiles x
    2 token chunks) alternating {0-3}/{4-7}; shared passes use 2-bank
    sets rotating through all 8.
  - (measured dead ends: fp8 DoubleRow would halve PE time but fails
    the 2e-2 gate — 6.5e-2 full-fp8, 3.8e-2 even B-GEMM-only; the
    Block postamble resets a fixed ~51-semaphore range per engine, so
    shrinking the kernel's semaphore count does not shrink it.)

Engine roles:
  sync  (SP) : weight DMAs + shared-B output DMAs
  tensor(PE) : all matmuls (bf16, 1 col/cycle)
  scalar(ACT): x k-tile loads; silu eviction PSUM->SBUF; routed-B
               output DMAs
  vector(DVE): g = silu(h1)*h3 multiply; PSUM->SBUF output copies
"""

from contextlib import ExitStack

import numpy as np

import concourse.bass as bass
import concourse.mybir as mybir

DIM = 1024
HIDDEN = 1024
NUM_EXPERTS = 8
TOP_K = 2
N_CORES = 8
P = 128
KT = DIM // P

DT = mybir.dt.bfloat16
W_SLOTS = 4   # weight-block ring (each slot holds up to [128, 8*512])
S_RING = 4    # silu scratch ring
N_WARMUP = 34  # PE warm-up matmuls: ~3.6 us of sustained PE activity,
               # enough to cover the HAM 4096-cycle activity window so
               # the 1.2->2.4 GHz clock gate opens before real matmuls
ACT_FUNC = mybir.ActivationFunctionType.Silu


def _chunks(total, maxc=512):
    """Split `total` into nearly-equal chunks of at most `maxc` columns."""
    if total <= maxc:
        return [(0, total)]
    n = (total + maxc - 1) // maxc
    h = ((total + n - 1) // n + 15) // 16 * 16
    out, off = [], 0
    while off + h < total:
        out.append((off, h))
        off += h
    out.append((off, total - off))
    return out


class Plan:
    """Per-engine instruction streams with planned semaphore counters."""

    ENGINES = ("sync", "tensor", "scalar", "vector", "gpsimd")

    def __init__(self):
        self.streams = {e: [] for e in self.ENGINES}
        self.cnt = {}
        self._waited = {}

    def wait(self, eng, sem, val):
        val = int(val)
        if val <= 0 or self._waited.get((eng, sem), 0) >= val:
            return
        self._waited[(eng, sem)] = val
        self.streams[eng].append(("wait", sem, val))

    def op(self, eng, fn, incs=()):
        self.streams[eng].append(("op", fn, tuple(incs)))
        for s, v in incs:
            self.cnt[s] = self.cnt.get(s, 0) + v


def build_program(C, S, act_func=ACT_FUNC, repeat=1):
    XW = C + S
    rch = _chunks(C)          # routed token chunks (local cols)
    sch = _chunks(S)          # shared token chunks
    assert len(rch) <= 2 and len(sch) <= 2

    nc = bass.Bass()
    tens = {}
    tens["xT"] = nc.declare_dram_parameter("xT", [DIM, XW], DT, isOutput=False)
    tens["w13"] = nc.declare_dram_parameter("w13", [DIM, 2 * HIDDEN], DT,
                                            isOutput=False)
    tens["w2"] = nc.declare_dram_parameter("w2", [HIDDEN, DIM], DT,
                                           isOutput=False)
    tens["w13s"] = nc.declare_dram_parameter("w13s", [DIM, 2 * HIDDEN], DT,
                                             isOutput=False)
    tens["w2s"] = nc.declare_dram_parameter("w2s", [HIDDEN, DIM], DT,
                                            isOutput=False)
    tens["yT"] = nc.declare_dram_parameter("yT", [DIM, XW], DT, isOutput=True)

    cwmax = max(cw for _, cw in rch + sch)

    # ---- weight block table ----------------------------------------
    # Ring blocks b = 0..11, slot = b % W_SLOTS, sem w{b % W_SLOTS}.
    # (name, m0, mcols); consuming passes are listed per block.  The
    # first routed-A block (pass 0, cols 0-255 of w13) lives in its own
    # wtk tensor, loaded per-k-tile (sems wk0-7) so k-tile 0 lands
    # ~0.7 us after issue and the PE starts as soon as x k-tile 0 does.
    # (Splitting the 512-col blocks further is a measured dead end: the
    # head supply pipe is saturated, so any block moved earlier pushes
    # an equal stall onto the next pass.)
    #   passes 0-7   routed A (w13, 256 cols per pass)
    #   passes 8-15  shared A (w13s)
    #   passes 16-23 routed B (w2, 128-col m-tile per pass)
    #   passes 24-31 shared B (w2s)
    blocks = [
        ("w13", 256, 256, [1]),
        ("w13", 512, 512, [2, 3]),
        ("w13", 1024, 512, [4, 5]),
        ("w13", 1536, 512, [6, 7]),
        ("w13s", 0, 512, [8, 9]),
        ("w13s", 512, 512, [10, 11]),
        ("w13s", 1024, 512, [12, 13]),
        ("w13s", 1536, 512, [14, 15]),
        ("w2", 0, 512, [16, 17, 18, 19]),
        ("w2", 512, 512, [20, 21, 22, 23]),
        ("w2s", 0, 512, [24, 25, 26, 27]),
        ("w2s", 512, 512, [28, 29, 30, 31]),
    ]
    # pass -> (block, m-offset in 128-col units within the block)
    pass_block = {}
    for b, (_nm, m0, mc, passes) in enumerate(blocks):
        for i, p in enumerate(passes):
            if p < 16:          # A pass: consumes 256 cols (2 m-tiles)
                pass_block[p] = (b, 2 * i)
            else:               # B pass: consumes 128 cols (1 m-tile)
                pass_block[p] = (b, i)

    plan = Plan()
    st = {"pass_idx": 0, "s_idx": 0,
          "bank_rel": [None] * 8, "s_rel": [None] * S_RING,
          "o_rel": {}, "od_cnt": {}}

    # per-slot release pass: block freed once its last consumer completes
    slot_free = [0] * W_SLOTS

    def weight_dma(b):
        """Issue ring block b ([1024, mc] -> slot [128, KT*mc]) on sync."""
        nm, m0, mc, passes = blocks[b]
        slot = b % W_SLOTS
        plan.wait("sync", "mm", slot_free[slot])

        def fn(e, _slot=slot, _nm=nm, _m0=m0, _mc=mc):
            src = tens[_nm][:, _m0:_m0 + _mc].rearrange(
                "(kk p) c -> p kk c", kk=KT)
            return e.dma_start(out=tens[f"wt{_slot}"][:, :KT * _mc], in_=src)

        wsem = f"w{slot}"
        plan.op("sync", fn, incs=((wsem, 16),))
        slot_free[slot] = max(passes) + 1
        return wsem, plan.cnt[wsem]

    def pe_pass(wtname, mc, moff, wsem_vals, rhs_base, chunks, bankset,
                g_wait=None, first=False, n_ml=2):
        """The 8-k-tile matmul burst of one pass (n_ml m-tiles x chunks).

        wtname/mc/moff: weight tensor, its block width per k-tile, and the
        first 128-col m-sub-tile this pass covers.  wsem_vals: list of
        (sem, val) waits, or per-kk callable when first=True."""
        p = st["pass_idx"]
        if not first:
            for sv in wsem_vals:
                plan.wait("tensor", *sv)
            for kk in range(KT):
                plan.wait("tensor", f"x{kk}", 16)
        ncn = len(chunks)
        n_mm = KT * n_ml * ncn
        i = 0
        for kk in range(KT):
            if first:
                if kk:
                    # garbage matmuls into bank 7 (pass 1 reopens it with
                    # start=True) keep the PE busy across the x-supply
                    # bubbles so the HAM clock gate opens at ~11 us
                    for _ in range(3):
                        def wmm(e):
                            return e.matmul(tens["pb7"][:, :P],
                                            lhsT=tens["wt0"][:, :P],
                                            rhs=tens["wt1"][:, :P],
                                            start=True, stop=True,
                                            skip_group_check=True)

                        plan.op("tensor", wmm)
                plan.wait("tensor", f"x{kk}", 16)
                plan.wait("tensor", f"wk{kk}", 16)
            if g_wait is not None:
                plan.wait("tensor", "g", g_wait(kk))
            for ml in range(n_ml):
                for ci, (c0, cw) in enumerate(chunks):
                    b = bankset[ml * ncn + ci]
                    if kk == 0 and st["bank_rel"][b] is not None:
                        rs, rv = st["bank_rel"][b]
                        plan.wait("tensor", rs, rv)
                    i += 1
                    incs = (("mm", 1),) if i == n_mm else ()

                    def mmop(e, _b=b, _wt=wtname, _kk=kk, _ml=moff + ml,
                             _c0=c0, _cw=cw, _rb=rhs_base, _mc=mc):
                        return e.matmul(
                            tens[f"pb{_b}"][:, :_cw],
                            lhsT=tens[_wt][:, _kk * _mc + _ml * P:
                                           _kk * _mc + (_ml + 1) * P],
                            rhs=tens["xg"][:, _kk * XW + _rb + _c0:
                                           _kk * XW + _rb + _c0 + _cw],
                            start=(_kk == 0), stop=(_kk == KT - 1),
                            skip_group_check=True)

                    plan.op("tensor", mmop, incs=incs)
        st["pass_idx"] += 1

    def a_tail(j, xbase, chunks, bankset):
        """Post-matmul work of A-pass j: silu on ACT, g-mul on DVE."""
        pdone = st["pass_idx"]  # mm value when this pass completes
        ncn = len(chunks)
        for ci, (c0, cw) in enumerate(chunks):
            st["s_idx"] += 1
            s_slot = st["s_idx"] % S_RING
            plan.wait("scalar", "mm", pdone)
            if st["s_rel"][s_slot] is not None:
                rs, rv = st["s_rel"][s_slot]
                plan.wait("scalar", rs, rv)

            def silu(e, _s=s_slot, _b=bankset[ci], _cw=cw):
                return e.activation(tens[f"s{_s}"][:, :_cw],
                                    tens[f"pb{_b}"][:, :_cw], act_func)

            plan.op("scalar", silu, incs=(("s", 1),))
            st["bank_rel"][bankset[ci]] = ("s", plan.cnt["s"])
            s_need = plan.cnt["s"]
            plan.wait("vector", "mm", pdone)
            plan.wait("vector", "s", s_need)
            b3 = bankset[ncn + ci]

            def mul(e, _j=j, _s=s_slot, _b=b3, _xb=xbase, _c0=c0, _cw=cw):
                col = _j * XW + GHALF + _xb + _c0
                return e.tensor_mul(tens["xg"][:, col:col + _cw],
                                    tens[f"s{_s}"][:, :_cw],
                                    tens[f"pb{_b}"][:, :_cw])

            plan.op("vector", mul, incs=(("g", 1),))
            st["bank_rel"][b3] = ("g", plan.cnt["g"])
            st["s_rel"][s_slot] = ("g", plan.cnt["g"])

    def b_tail(m, bankset, chunks, otile, odsem, odeng, ybase,
               pipeline=False):
        """Post-matmul work of B-pass for m-tile m: copy on DVE, output
        DMA on `odeng` with sem `odsem` (one in-flight DMA per sem).
        pipeline=True (single-bank pass only): split the output in two
        halves with per-half copy->DMA pipelining, shortening the
        end-of-program drain chain on the very last pass."""
        pdone = st["pass_idx"]
        plan.wait("vector", "mm", pdone)
        key = (otile, odsem)
        if key in st["o_rel"]:
            plan.wait("vector", odsem, st["o_rel"][key])

        def odma_piece(c0, cw):
            st["od_cnt"][odsem] = st["od_cnt"].get(odsem, 0) + 16

            def odma(e, _o=otile, _m=m, _yb=ybase + c0, _c0=c0, _cw=cw):
                return e.dma_start(
                    out=tens["yT"][_m * P:(_m + 1) * P, _yb:_yb + _cw],
                    in_=tens[f"ot{_o}"][:, _c0:_c0 + _cw])

            plan.op(odeng, odma, incs=((odsem, 16),))

        if not pipeline:
            for ci, (c0, cw) in enumerate(chunks):
                b = bankset[ci]

                def cp(e, _o=otile, _b=b, _c0=c0, _cw=cw):
                    return e.tensor_copy(tens[f"ot{_o}"][:, _c0:_c0 + _cw],
                                         tens[f"pb{_b}"][:, :_cw])

                plan.op("vector", cp, incs=(("o", 1),))
                st["bank_rel"][b] = ("o", plan.cnt["o"])
            span = chunks[-1][0] + chunks[-1][1]
            plan.wait(odeng, "o", plan.cnt["o"])
            odma_piece(0, span)
        else:
            assert len(chunks) == 1
            b = bankset[0]
            total = chunks[0][1]
            h = total // 2
            for c0, cw in ((0, h), (h, total - h)):
                def cp(e, _o=otile, _b=b, _c0=c0, _cw=cw):
                    return e.tensor_copy(tens[f"ot{_o}"][:, _c0:_c0 + _cw],
                                         tens[f"pb{_b}"][:, _c0:_c0 + _cw])

                plan.op("vector", cp, incs=(("o", 1),))
                plan.wait(odeng, "o", plan.cnt["o"])
                odma_piece(c0, cw)
            st["bank_rel"][b] = ("o", plan.cnt["o"])
        st["o_rel"][key] = st["od_cnt"][odsem]

    with ExitStack() as ctx:
        def sb(name, shape, dt):
            tens[name] = ctx.enter_context(nc.sbuf_tensor(name, shape, dt))

        # xg holds x tiles (first KT*XW cols) and g tiles (next KT*XW)
        GHALF = KT * XW
        sb("xg", [P, 2 * KT * XW], DT)
        sb("wtk", [P, KT * 256], DT)
        for r in range(W_SLOTS):
            sb(f"wt{r}", [P, KT * 512], DT)
        for r in range(S_RING):
            sb(f"s{r}", [P, cwmax], DT)
        for r in range(4):
            sb(f"ot{r}", [P, C], DT)
        for r in range(4, 8):
            sb(f"ot{r}", [P, S], DT)
        for b in range(8):
            tens[f"pb{b}"] = ctx.enter_context(
                nc.psum_tensor(f"pb{b}", [P, 512], mybir.dt.float32))

        r4 = [(0, 1, 2, 3), (4, 5, 6, 7)]       # routed 4-bank sets
        s2 = [(0, 1), (2, 3), (4, 5), (6, 7)]   # shared 2-bank sets

        # ---- PE warm-up: garbage matmuls, results discarded ----------
        for _ in range(N_WARMUP):
            def wmm(e):
                return e.matmul(tens["pb7"][:, :P],
                                lhsT=tens["wt0"][:, :P],
                                rhs=tens["wt1"][:, :P],
                                start=True, stop=True,
                                skip_group_check=True)

            plan.op("tensor", wmm)

        for _rep in range(repeat):
            g0 = plan.cnt.get("g", 0)

            # -- x loads: all on the scalar HWDGE queue ----------------
            for kk in range(KT):
                def xl(e, _kk=kk):
                    return e.dma_start(
                        out=tens["xg"][:, _kk * XW:(_kk + 1) * XW],
                        in_=tens["xT"][_kk * P:(_kk + 1) * P, :])

                plan.op("scalar", xl, incs=((f"x{kk}", 16),))

            # -- first w13 block (cols 0-255): per-k-tile on sync so the
            # -- PE can start on k-tile 0 ~0.7 us after issue; the pass-1
            # -- block (b0) is prefetched right after k-tile 0 ----------
            wv = {}
            for kk in range(KT):
                def wl(e, _kk=kk):
                    return e.dma_start(
                        out=tens["wtk"][:, _kk * 256:(_kk + 1) * 256],
                        in_=tens["w13"][_kk * P:(_kk + 1) * P, 0:256])

                plan.op("sync", wl, incs=((f"wk{kk}", 16),))

            # -- prefetch ring blocks as slots free (after the wk tiles:
            # -- the head is HBM-bandwidth-bound, so blocks are ordered
            # -- strictly by first-need time) --------------------------
            for b in range(len(blocks)):
                wv[b] = weight_dma(b)

            # -- routed A: 8 passes ------------------------------------
            pe_pass("wtk", 256, 0, None, 0, rch, r4[0], first=True)
            a_tail(0, 0, rch, r4[0])
            for j in range(1, KT):
                b, moff = pass_block[j]
                pe_pass(f"wt{b % W_SLOTS}", blocks[b][2], moff, [wv[b]],
                        0, rch, r4[j % 2])
                a_tail(j, 0, rch, r4[j % 2])
            g_routed_end = plan.cnt.get("g", 0)

            # -- shared A: 8 passes ------------------------------------
            for j in range(KT):
                p = 8 + j
                b, moff = pass_block[p]
                pe_pass(f"wt{b % W_SLOTS}", blocks[b][2], moff, [wv[b]],
                        C, sch, s2[j % 4])
                a_tail(j, C, sch, s2[j % 4])

            # -- routed B: 8 m-passes, output DMAs on scalar -----------
            for m in range(KT):
                p = 16 + m
                b, moff = pass_block[p]
                pe_pass(f"wt{b % W_SLOTS}", blocks[b][2], moff, [wv[b]],
                        GHALF, rch, s2[m % 4],
                        g_wait=lambda kk, _g0=g0: _g0 + len(rch) * (kk + 1),
                        n_ml=1)
                b_tail(m, s2[m % 4], rch, m % 4, f"odr{m % 4}", "scalar", 0)

            # -- shared B: 8 m-passes, output DMAs on sync -------------
            for m in range(KT):
                p = 24 + m
                b, moff = pass_block[p]
                pe_pass(f"wt{b % W_SLOTS}", blocks[b][2], moff, [wv[b]],
                        GHALF + C, sch, (m % 8,),
                        g_wait=lambda kk, _g0=g_routed_end: _g0 + kk + 1,
                        n_ml=1)
                b_tail(m, (m % 8,), sch, 4 + m % 4, f"ods{m % 4}", "sync", C)

        # ---- drain: wait for the last output DMAs on their queues ----
        for i in range(4):
            if plan.cnt.get(f"odr{i}", 0):
                plan.wait("scalar", f"odr{i}", plan.cnt[f"odr{i}"])
            if plan.cnt.get(f"ods{i}", 0):
                plan.wait("sync", f"ods{i}", plan.cnt[f"ods{i}"])

        # ---- emit ----
        with ExitStack() as sem_ctx:
            sems = {}
            for name in plan.cnt:
                sems[name] = sem_ctx.enter_context(nc.semaphore(f"sem_{name}"))

            with nc.Block() as block:
                def runner(stream):
                    def run(e):
                        for item in stream:
                            if item[0] == "wait":
                                _, s, v = item
                                e.wait_ge(sems[s], v)
                            else:
                                _, fn, incs = item
                                inst = fn(e)
                                rest = list(incs)
                                if rest and inst is not None:
                                    s, v = rest.pop(0)
                                    inst.then_inc(sems[s], v)
                                for s, v in rest:
                                    e.sem_inc(sems[s], v)
                    return run

                block.sync(runner(plan.streams["sync"]))
                block.tensor(runner(plan.streams["tensor"]))
                block.scalar(runner(plan.streams["scalar"]))
                block.vector(runner(plan.streams["vector"]))
                if plan.streams["gpsimd"]:
                    block.gpsimd(runner(plan.streams["gpsimd"]))
    return nc


def _interleave_w13(w1e, w3e, np_dt):
    out = np.empty((DIM, 2 * HIDDEN), dtype=np.float32)
    v = out.reshape(DIM, HIDDEN // P, 2, P)
    v[:, :, 0, :] = w1e.reshape(DIM, HIDDEN // P, P)
    v[:, :, 1, :] = w3e.reshape(DIM, HIDDEN // P, P)
    return out.astype(np_dt)


def route(xt, gate_w):
    logits = (xt @ gate_w.T).astype(np.float32)
    m = logits.max(axis=1, keepdims=True)
    e = np.exp(logits - m)
    scores = (e / e.sum(axis=1, keepdims=True)).astype(np.float32)
    sel = np.argsort(-scores, axis=1, kind="stable")[:, :TOP_K].astype(np.int32)
    top_scores = np.take_along_axis(scores, sel, axis=1)
    sel_flat = sel.reshape(-1)
    order = np.argsort(sel_flat, kind="stable")
    token_idx = (order // TOP_K).astype(np.int64)
    eid = sel_flat[order]
    scores_sorted = top_scores.reshape(-1)[order]
    return token_idx, eid, scores_sorted


def kernel(x, gate_w, w1, w2, w3, w1s, w2s, w3s, _run=None):
    x = np.asarray(x, dtype=np.float32)
    bs, slen, dim = x.shape
    N = bs * slen
    xt = np.ascontiguousarray(x.reshape(N, dim))
    S = N // N_CORES

    token_idx, eid, scores_sorted = route(xt, np.asarray(gate_w, np.float32))

    counts = np.bincount(eid, minlength=NUM_EXPERTS)
    C = int(max(256, ((counts.max() + 7) // 8) * 8))

    np_dt = mybir.dt.np(DT)
    bounds = np.concatenate([[0], np.cumsum(counts)])
    w13s_i = _interleave_w13(np.asarray(w1s[0], np.float32),
                             np.asarray(w3s[0], np.float32), np_dt)
    w2s_c = np.asarray(w2s[0], np.float32).astype(np_dt)

    in_maps = []
    tok_per_core = []
    for e2 in range(N_CORES):
        lo, hi = int(bounds[e2]), int(bounds[e2 + 1])
        toks = token_idx[lo:hi]
        tok_per_core.append(toks)
        xfull = np.zeros((C + S, dim), np.float32)
        xfull[: hi - lo] = xt[toks] * scores_sorted[lo:hi, None]
        xfull[C:] = xt[e2 * S:(e2 + 1) * S]
        in_maps.append({
            "xT": np.ascontiguousarray(xfull.T).astype(np_dt),
            "w13": _interleave_w13(np.asarray(w1[e2], np.float32),
                                   np.asarray(w3[e2], np.float32), np_dt),
            "w2": np.asarray(w2[e2], np.float32).astype(np_dt),
            "w13s": w13s_i,
            "w2s": w2s_c,
        })

    nc = build_program(C, S)
    if _run is None:
        from concourse.bass_utils import run_bass_kernel_spmd
        results = run_bass_kernel_spmd(nc, in_maps, list(range(N_CORES))).results
    else:
        results = _run(nc, in_maps)

    out = np.empty((N, dim), np.float32)
    for e2 in range(N_CORES):
        y = np.asarray(results[e2]["yT"], dtype=np.float32)
        out[e2 * S:(e2 + 1) * S] = y[:, C:].T
    for e2 in range(N_CORES):
        cnt = len(tok_per_core[e2])
        out[tok_per_core[e2]] += np.asarray(
            results[e2]["yT"][:, :cnt], dtype=np.float32).T
    return out.reshape(bs, slen, dim)
